# revision 7
# baseline (speedup 1.0000x reference)
"""Multi-head causal self-attention on 8 trn2 NeuronCores.

Sharding: data-parallel over batch (B=2) x tensor-parallel over heads
(16 heads -> 4 per core). Each core computes, for its (batch, 4-head
group): QKV projections (rows of Wq/Wk/Wv), causal attention, and a
partial output projection against its column slice of Wo. The host
sums the 4 partials per batch and adds bo.

Device kernel layout notes (per core):
  xT    : x^T         [D(part) x S]   8 tiles [128, 2048]
  qT/kT : (head dims)(part) x S, 2 tiles [128, 2048] (2 heads each)
  v_aug : [S(part) x (4*65)]  v columns + ones column per head (the
          ones column makes the attn@V matmul also emit the softmax
          denominator as output row 64)
  scoresT[t, s] = K Q^T blocks -> exp on ACT -> attn@V accumulation.
  Causal: only lower (t <= s) blocks are computed; diagonal blocks are
  masked with an on-device upper-triangular multiplicative mask.
All matmuls run as float32r (full-rate fp32 PE path, N>=256).
"""

import numpy as np

import concourse.bass as bass
import concourse.mybir as mybir
import concourse.tile as tile
from concourse import bacc
from concourse.bass_utils import run_bass_kernel_spmd
from concourse.masks import make_identity, make_upper_triangular

B, S, D, H = 2, 2048, 1024, 16
DK = D // H           # 64
HL = 4                # heads per core
RL = HL * DK          # 256 local head-dim rows
N_CORES = 8

f32 = mybir.dt.float32
f32r = mybir.dt.float32r
EXP = mybir.ActivationFunctionType.Exp
SCALE = 1.0 / np.sqrt(DK).astype(np.float32).item()

_CACHE: dict = {}


def _build_nc():
    from contextlib import ExitStack

    nc = bacc.Bacc("TRN2", target_bir_lowering=False, debug=False)

    xb = nc.dram_tensor("xb", [S, D], f32, kind="ExternalInput").ap()
    wq = nc.dram_tensor("wq", [RL, D], f32, kind="ExternalInput").ap()
    wk = nc.dram_tensor("wk", [RL, D], f32, kind="ExternalInput").ap()
    wv = nc.dram_tensor("wv", [RL, D], f32, kind="ExternalInput").ap()
    wo = nc.dram_tensor("wo", [D, RL], f32, kind="ExternalInput").ap()
    bq = nc.dram_tensor("bq", [RL], f32, kind="ExternalInput").ap()
    bk = nc.dram_tensor("bk", [RL], f32, kind="ExternalInput").ap()
    bv = nc.dram_tensor("bv", [RL], f32, kind="ExternalInput").ap()
    outp = nc.dram_tensor("outp", [S, D], f32, kind="ExternalOutput").ap()

    with tile.TileContext(nc) as tc, ExitStack() as ctx:
        const = ctx.enter_context(tc.tile_pool(name="const", bufs=1))
        ident = const.tile([128, 128], f32)
        make_identity(nc, ident)
        tri = const.tile([128, 128], f32)
        make_upper_triangular(nc, tri, val=1.0, diag=True)
        ones = const.tile([1, 512], f32r)
        nc.vector.memset(ones.bitcast(f32), 1.0)
        bq_sb = const.tile([1, RL], f32r)
        bk_sb = const.tile([1, RL], f32r)
        bv_sb = const.tile([1, RL], f32r)
        for b_dram, b_sb, nm in ((bq, bq_sb, "blq"), (bk, bk_sb, "blk"),
                                 (bv, bv_sb, "blv")):
            bl = const.tile([1, RL], f32, name=nm)
            nc.sync.dma_start(out=bl, in_=b_dram.rearrange("(a b) -> a b", a=1))
            nc.vector.tensor_copy(b_sb, bl)

        pers = ctx.enter_context(tc.tile_pool(name="pers", bufs=1))
        qT = [pers.tile([128, S], f32r, tag=f"qT{m}", name=f"qT{m}") for m in range(2)]
        kT = [pers.tile([128, S], f32r, tag=f"kT{m}", name=f"kT{m}") for m in range(2)]
        vaug = [pers.tile([128, HL * 65], f32r, tag=f"va{t}", name=f"va{t}") for t in range(16)]
        oT = [pers.tile([128, S], f32r, tag=f"oT{m}", name=f"oT{m}") for m in range(2)]
        woT = [pers.tile([128, D], f32r, tag=f"woT{r}", name=f"woT{r}") for r in range(2)]
        wqT = [pers.tile([128, RL], f32r, tag=f"wqT{k}", name=f"wqT{k}") for k in range(8)]
        wkT = [pers.tile([128, RL], f32r, tag=f"wkT{k}", name=f"wkT{k}") for k in range(8)]
        wvT = [pers.tile([128, RL], f32r, tag=f"wvT{k}", name=f"wvT{k}") for k in range(8)]

        # ---------------- phase 1-3: transposes + QKV projections ----------
        with tc.tile_pool(name="xin", bufs=3) as xin, \
             tc.tile_pool(name="win", bufs=2) as win, \
             tc.tile_pool(name="xTp", bufs=1) as xTp, \
             tc.tile_pool(name="tps", bufs=4, space="PSUM") as tps, \
             tc.tile_pool(name="mmp", bufs=4, space="PSUM") as mmp:
            xT = [xTp.tile([128, S], f32r, tag=f"xT{k}", name=f"xT{k}") for k in range(8)]

            for wdram, wTl in ((wq, wqT), (wk, wkT), (wv, wvT)):
                for mt in range(2):
                    wt = win.tile([128, D], f32, tag="wload")
                    nc.sync.dma_start(out=wt, in_=wdram[mt * 128:(mt + 1) * 128, :])
                    for k in range(8):
                        pt = tps.tile([128, 128], f32, tag="tp")
                        nc.tensor.transpose(pt, wt[:, k * 128:(k + 1) * 128], ident)
                        nc.vector.tensor_copy(wTl[k][:, mt * 128:(mt + 1) * 128], pt)
            for jt in range(8):
                wt = win.tile([128, RL], f32, tag="wload")
                nc.sync.dma_start(out=wt, in_=wo[jt * 128:(jt + 1) * 128, :])
                for rt in range(2):
                    pt = tps.tile([128, 128], f32, tag="tp")
                    nc.tensor.transpose(pt, wt[:, rt * 128:(rt + 1) * 128], ident)
                    nc.vector.tensor_copy(woT[rt][:, jt * 128:(jt + 1) * 128], pt)
            for st in range(16):
                xt_ = xin.tile([128, D], f32, tag="xload")
                nc.sync.dma_start(out=xt_, in_=xb[st * 128:(st + 1) * 128, :])
                for k in range(8):
                    pt = tps.tile([128, 128], f32, tag="tp")
                    nc.tensor.transpose(pt, xt_[:, k * 128:(k + 1) * 128], ident)
                    nc.vector.tensor_copy(xT[k][:, st * 128:(st + 1) * 128], pt)

            # qT / kT: out[m(=2 heads' dims), s] accumulated over k-tiles
            for wTl, b_sb, dst in ((wqT, bq_sb, qT), (wkT, bk_sb, kT)):
                for mt in range(2):
                    for sc in range(4):
                        ps = mmp.tile([128, 512], f32, tag="pqk")
                        for k in range(8):
                            nc.tensor.matmul(
                                ps,
                                wTl[k][:, mt * 128:(mt + 1) * 128],
                                xT[k][:, sc * 512:(sc + 1) * 512],
                                start=(k == 0), stop=False)
                        nc.tensor.matmul(
                            ps,
                            b_sb[:, mt * 128:(mt + 1) * 128],
                            ones,
                            start=False, stop=True)
                        nc.vector.tensor_copy(dst[mt][:, sc * 512:(sc + 1) * 512], ps)

            # v: out[s, r] accumulated over k-tiles; scattered into v_aug
            for st in range(16):
                ps = mmp.tile([128, RL], f32, tag="pqk", name="pv")
                for k in range(8):
                    nc.tensor.matmul(
                        ps,
                        xT[k][:, st * 128:(st + 1) * 128],
                        wvT[k],
                        start=(k == 0), stop=False)
                nc.tensor.matmul(
                    ps, ones[:, 0:128], bv_sb,
                    start=False, stop=True)
                va3 = vaug[st].rearrange("p (h e) -> p h e", e=65)
                nc.vector.memset(va3[:, :, 64:65].bitcast(f32), 1.0)
                nc.vector.tensor_copy(
                    va3[:, :, 0:64], ps.rearrange("p (h e) -> p h e", e=64))

        # ---------------- phase 4: attention ------------------------------
        with tc.tile_pool(name="expp", bufs=4) as expp, \
             tc.tile_pool(name="rowp", bufs=4) as rowp, \
             tc.tile_pool(name="drp", bufs=4, space="DRAM") as drp, \
             tc.tile_pool(name="obp", bufs=3) as obp, \
             tc.tile_pool(name="scp", bufs=3, space="PSUM") as scp, \
             tc.tile_pool(name="otp", bufs=2, space="PSUM") as otp, \
             tc.tile_pool(name="rpp", bufs=2, space="PSUM") as rpp:
            for hp in range(2):
                for j in range(4):
                    i_max = 4 * j + 3
                    pso = {0: otp.tile([65, 512], f32, tag="ot", name="otA"),
                           64: otp.tile([65, 512], f32, tag="ot", name="otB")}
                    for i in range(i_max + 1):
                        d = i - 4 * j  # >= 0: diagonal block index
                        for pb in (0, 64):
                            h_loc = 2 * hp + (pb // 64)
                            sc_ps = scp.tile([128, 512], f32, tag="sc")
                            nc.tensor.matmul(
                                sc_ps,
                                kT[hp][pb:pb + 64, i * 128:(i + 1) * 128],
                                qT[hp][pb:pb + 64, j * 512:(j + 1) * 512],
                                start=True, stop=True)
                            et = expp.tile([128, 512], f32r, tag="e")
                            if d < 0:
                                nc.scalar.activation(et, sc_ps, EXP, scale=SCALE)
                            else:
                                off = 128 * d
                                if off > 0:
                                    nc.gpsimd.memset(et[:, 0:off].bitcast(f32), 0.0)
                                nc.scalar.activation(
                                    et[:, off:512], sc_ps[:, off:512], EXP,
                                    scale=SCALE)
                                nc.vector.tensor_mul(
                                    et[:, off:off + 128], et[:, off:off + 128], tri)
                            nc.tensor.matmul(
                                pso[pb],
                                vaug[i][:, h_loc * 65:(h_loc + 1) * 65],
                                et,
                                start=(i == 0), stop=(i == i_max))
                    # softmax divide: row 64 of pso holds the denominators
                    for pb in (0, 64):
                        # reciprocal at the same partition (64), then bounce
                        # through DRAM to broadcast across 64 partitions
                        dn = rowp.tile([65, 512], f32, tag="dn")
                        nc.vector.reciprocal(dn[64:65, :], pso[pb][64:65, :])
                        dnd = drp.tile([1, 512], f32, tag="dnd")
                        nc.sync.dma_start(out=dnd, in_=dn[64:65, :])
                        rp = rowp.tile([64, 512], f32, tag="rp")
                        bcast = bass.AP(
                            tensor=dnd.tensor, offset=dnd.offset,
                            ap=[[0, 64]] + [list(p) for p in dnd.ap[1:]])
                        nc.sync.dma_start(out=rp, in_=bcast)
                        if pb == 0:
                            nc.vector.tensor_mul(
                                oT[hp][0:64, j * 512:(j + 1) * 512],
                                pso[pb][0:64, :], rp)
                        else:
                            ob = obp.tile([64, 512], f32r, tag="ob")
                            nc.vector.tensor_mul(ob, pso[pb][0:64, :], rp)
                            nc.sync.dma_start(
                                out=oT[hp][64:128, j * 512:(j + 1) * 512], in_=ob)

        # ---------------- phase 5: output projection -----------------------
        with tc.tile_pool(name="pop", bufs=4, space="PSUM") as pop, \
             tc.tile_pool(name="outs", bufs=4) as outs:
            for st in range(16):
                for jc in range(2):
                    ps = pop.tile([128, 512], f32, tag="po")
                    for rt in range(2):
                        nc.tensor.matmul(
                            ps,
                            oT[rt][:, st * 128:(st + 1) * 128],
                            woT[rt][:, jc * 512:(jc + 1) * 512],
                            start=(rt == 0), stop=(rt == 1))
                    ot_sb = outs.tile([128, 512], f32, tag="osb")
                    nc.vector.tensor_copy(ot_sb, ps)
                    nc.sync.dma_start(
                        out=outp[st * 128:(st + 1) * 128, jc * 512:(jc + 1) * 512],
                        in_=ot_sb)

    nc.compile()
    return nc


def _get_nc():
    if "nc" not in _CACHE:
        _CACHE["nc"] = _build_nc()
    return _CACHE["nc"]


def kernel(x, Wq, bq, Wk, bk, Wv, bv, Wo, bo, mask, _trace=False):
    x = np.asarray(x, dtype=np.float32)
    nc = _get_nc()
    in_maps = []
    for c in range(N_CORES):
        b, g = divmod(c, 4)
        r0 = g * RL
        in_maps.append({
            "xb": np.ascontiguousarray(x[b]),
            "wq": np.ascontiguousarray(Wq[r0:r0 + RL]),
            "wk": np.ascontiguousarray(Wk[r0:r0 + RL]),
            "wv": np.ascontiguousarray(Wv[r0:r0 + RL]),
            "wo": np.ascontiguousarray(Wo[:, r0:r0 + RL]),
            "bq": np.ascontiguousarray(bq[r0:r0 + RL]),
            "bk": np.ascontiguousarray(bk[r0:r0 + RL]),
            "bv": np.ascontiguousarray(bv[r0:r0 + RL]),
        })
    res = run_bass_kernel_spmd(nc, in_maps, core_ids=list(range(N_CORES)),
                               trace=_trace)
    out = np.zeros((B, S, D), dtype=np.float32)
    for b in range(B):
        acc = np.zeros((S, D), dtype=np.float32)
        for g in range(4):
            acc += res.results[4 * b + g]["outp"]
        out[b] = acc + np.asarray(bo, dtype=np.float32)[None, :]
    if _trace:
        return out, res
    return out


# revision 8
# speedup vs baseline: 1.1439x; 1.1439x over previous
"""Multi-head causal self-attention on 8 trn2 NeuronCores.

Sharding: data-parallel over batch (B=2) x tensor-parallel over heads
(16 heads -> 4 per core). Each core computes, for its (batch, 4-head
group): QKV projections (rows of Wq/Wk/Wv), causal attention, and a
partial output projection against its column slice of Wo. The host
sums the 4 partials per batch and adds bo.

Device kernel layout notes (per core):
  xT    : x^T         [D(part) x S]   8 tiles [128, 2048]
  qT/kT : (head dims)(part) x S, 2 tiles [128, 2048] (2 heads each)
  v_aug : [S(part) x (4*65)]  v columns + ones column per head (the
          ones column makes the attn@V matmul also emit the softmax
          denominator as output row 64)
  scoresT[t, s] = K Q^T blocks -> exp on ACT -> attn@V accumulation.
  Causal: only lower (t <= s) blocks are computed; diagonal blocks are
  masked with an on-device upper-triangular multiplicative mask.
All matmuls run as float32r (full-rate fp32 PE path, N>=256).
"""

import numpy as np

import concourse.bass as bass
import concourse.mybir as mybir
import concourse.tile as tile
from concourse import bacc
from concourse.bass_utils import run_bass_kernel_spmd
from concourse.masks import make_identity, make_upper_triangular

B, S, D, H = 2, 2048, 1024, 16
DK = D // H           # 64
HL = 4                # heads per core
RL = HL * DK          # 256 local head-dim rows
N_CORES = 8

f32 = mybir.dt.float32
f32r = mybir.dt.float32r
bf = mybir.dt.bfloat16
EXP = mybir.ActivationFunctionType.Exp
SCALE = 1.0 / np.sqrt(DK).astype(np.float32).item()

_CACHE: dict = {}


def _build_nc():
    from contextlib import ExitStack

    nc = bacc.Bacc("TRN2", target_bir_lowering=False, debug=False)

    xb = nc.dram_tensor("xb", [S, D], f32, kind="ExternalInput").ap()
    wq = nc.dram_tensor("wq", [RL, D], f32, kind="ExternalInput").ap()
    wk = nc.dram_tensor("wk", [RL, D], f32, kind="ExternalInput").ap()
    wv = nc.dram_tensor("wv", [RL, D], f32, kind="ExternalInput").ap()
    wo = nc.dram_tensor("wo", [D, RL], f32, kind="ExternalInput").ap()
    bq = nc.dram_tensor("bq", [RL], f32, kind="ExternalInput").ap()
    bk = nc.dram_tensor("bk", [RL], f32, kind="ExternalInput").ap()
    bv = nc.dram_tensor("bv", [RL], f32, kind="ExternalInput").ap()
    outp = nc.dram_tensor("outp", [S, D], f32, kind="ExternalOutput").ap()

    with tile.TileContext(nc) as tc, ExitStack() as ctx:
        const = ctx.enter_context(tc.tile_pool(name="const", bufs=1))
        ident = const.tile([128, 128], f32)
        make_identity(nc, ident)
        tri = const.tile([128, 128], bf)
        make_upper_triangular(nc, tri, val=1.0, diag=True)
        ones = const.tile([1, 512], bf)
        nc.vector.memset(ones, 1.0)
        bq_sb = const.tile([1, RL], bf)
        bk_sb = const.tile([1, RL], bf)
        bv_sb = const.tile([1, RL], bf)
        for b_dram, b_sb, nm in ((bq, bq_sb, "blq"), (bk, bk_sb, "blk"),
                                 (bv, bv_sb, "blv")):
            bl = const.tile([1, RL], f32, name=nm)
            nc.sync.dma_start(out=bl, in_=b_dram.rearrange("(a b) -> a b", a=1))
            nc.vector.tensor_copy(b_sb, bl)

        pers = ctx.enter_context(tc.tile_pool(name="pers", bufs=1))
        qT = [pers.tile([128, S], bf, tag=f"qT{m}", name=f"qT{m}") for m in range(2)]
        kT = [pers.tile([128, S], bf, tag=f"kT{m}", name=f"kT{m}") for m in range(2)]
        vaug = [pers.tile([128, HL * 65], bf, tag=f"va{t}", name=f"va{t}") for t in range(16)]
        oT = [pers.tile([128, S], bf, tag=f"oT{m}", name=f"oT{m}") for m in range(2)]
        woT = [pers.tile([128, D], bf, tag=f"woT{r}", name=f"woT{r}") for r in range(2)]
        wqT = [pers.tile([128, RL], bf, tag=f"wqT{k}", name=f"wqT{k}") for k in range(8)]
        wkT = [pers.tile([128, RL], bf, tag=f"wkT{k}", name=f"wkT{k}") for k in range(8)]
        wvT = [pers.tile([128, RL], bf, tag=f"wvT{k}", name=f"wvT{k}") for k in range(8)]

        # ---------------- phase 1-3: transposes + QKV projections ----------
        with tc.tile_pool(name="xin", bufs=3) as xin, \
             tc.tile_pool(name="win", bufs=2) as win, \
             tc.tile_pool(name="xTp", bufs=1) as xTp, \
             tc.tile_pool(name="tps", bufs=4, space="PSUM") as tps, \
             tc.tile_pool(name="mmp", bufs=4, space="PSUM") as mmp:
            xT = [xTp.tile([128, S], bf, tag=f"xT{k}", name=f"xT{k}") for k in range(8)]

            for wdram, wTl in ((wq, wqT), (wk, wkT), (wv, wvT)):
                for mt in range(2):
                    wt = win.tile([128, D], f32, tag="wload")
                    nc.sync.dma_start(out=wt, in_=wdram[mt * 128:(mt + 1) * 128, :])
                    for k in range(8):
                        pt = tps.tile([128, 128], f32, tag="tp")
                        nc.tensor.transpose(pt, wt[:, k * 128:(k + 1) * 128], ident)
                        nc.vector.tensor_copy(wTl[k][:, mt * 128:(mt + 1) * 128], pt)
            for jt in range(8):
                wt = win.tile([128, RL], f32, tag="wload")
                nc.sync.dma_start(out=wt, in_=wo[jt * 128:(jt + 1) * 128, :])
                for rt in range(2):
                    pt = tps.tile([128, 128], f32, tag="tp")
                    nc.tensor.transpose(pt, wt[:, rt * 128:(rt + 1) * 128], ident)
                    nc.vector.tensor_copy(woT[rt][:, jt * 128:(jt + 1) * 128], pt)
            for st in range(16):
                xt_ = xin.tile([128, D], f32, tag="xload")
                nc.sync.dma_start(out=xt_, in_=xb[st * 128:(st + 1) * 128, :])
                for k in range(8):
                    pt = tps.tile([128, 128], f32, tag="tp")
                    nc.tensor.transpose(pt, xt_[:, k * 128:(k + 1) * 128], ident)
                    nc.vector.tensor_copy(xT[k][:, st * 128:(st + 1) * 128], pt)

            # qT / kT: out[m(=2 heads' dims), s] accumulated over k-tiles
            for wTl, b_sb, dst in ((wqT, bq_sb, qT), (wkT, bk_sb, kT)):
                for mt in range(2):
                    for sc in range(4):
                        ps = mmp.tile([128, 512], f32, tag="pqk")
                        for k in range(8):
                            nc.tensor.matmul(
                                ps,
                                wTl[k][:, mt * 128:(mt + 1) * 128],
                                xT[k][:, sc * 512:(sc + 1) * 512],
                                start=(k == 0), stop=False)
                        nc.tensor.matmul(
                            ps,
                            b_sb[:, mt * 128:(mt + 1) * 128],
                            ones,
                            start=False, stop=True)
                        nc.scalar.copy(dst[mt][:, sc * 512:(sc + 1) * 512], ps)

            # v: out[s, r] accumulated over k-tiles; scattered into v_aug
            for st in range(16):
                ps = mmp.tile([128, RL], f32, tag="pqk", name="pv")
                for k in range(8):
                    nc.tensor.matmul(
                        ps,
                        xT[k][:, st * 128:(st + 1) * 128],
                        wvT[k],
                        start=(k == 0), stop=False)
                nc.tensor.matmul(
                    ps, ones[:, 0:128], bv_sb,
                    start=False, stop=True)
                va3 = vaug[st].rearrange("p (h e) -> p h e", e=65)
                nc.vector.memset(va3[:, :, 64:65], 1.0)
                nc.scalar.copy(
                    va3[:, :, 0:64], ps.rearrange("p (h e) -> p h e", e=64))

        # ---------------- phase 4: attention ------------------------------
        with tc.tile_pool(name="expp", bufs=4) as expp, \
             tc.tile_pool(name="rowp", bufs=4) as rowp, \
             tc.tile_pool(name="drp", bufs=4, space="DRAM") as drp, \
             tc.tile_pool(name="obp", bufs=3) as obp, \
             tc.tile_pool(name="scp", bufs=3, space="PSUM") as scp, \
             tc.tile_pool(name="otp", bufs=2, space="PSUM") as otp, \
             tc.tile_pool(name="rpp", bufs=2, space="PSUM") as rpp:
            for hp in range(2):
                for j in range(4):
                    i_max = 4 * j + 3
                    pso = {0: otp.tile([65, 512], f32, tag="ot", name="otA"),
                           64: otp.tile([65, 512], f32, tag="ot", name="otB")}
                    for i in range(i_max + 1):
                        d = i - 4 * j  # >= 0: diagonal block index
                        for pb in (0, 64):
                            h_loc = 2 * hp + (pb // 64)
                            sc_ps = scp.tile([128, 512], f32, tag="sc")
                            nc.tensor.matmul(
                                sc_ps,
                                kT[hp][pb:pb + 64, i * 128:(i + 1) * 128],
                                qT[hp][pb:pb + 64, j * 512:(j + 1) * 512],
                                start=True, stop=True)
                            et = expp.tile([128, 512], bf, tag="e")
                            if d < 0:
                                nc.scalar.activation(et, sc_ps, EXP, scale=SCALE)
                            else:
                                off = 128 * d
                                if off > 0:
                                    nc.gpsimd.memset(et[:, 0:off], 0.0)
                                nc.scalar.activation(
                                    et[:, off:512], sc_ps[:, off:512], EXP,
                                    scale=SCALE)
                                nc.vector.tensor_mul(
                                    et[:, off:off + 128], et[:, off:off + 128], tri)
                            nc.tensor.matmul(
                                pso[pb],
                                vaug[i][:, h_loc * 65:(h_loc + 1) * 65],
                                et,
                                start=(i == 0), stop=(i == i_max))
                    # softmax divide: row 64 of pso holds the denominators
                    for pb in (0, 64):
                        # reciprocal at the same partition (64), then bounce
                        # through DRAM to broadcast across 64 partitions
                        dn = rowp.tile([65, 512], f32, tag="dn")
                        nc.vector.reciprocal(dn[64:65, :], pso[pb][64:65, :])
                        dnd = drp.tile([1, 512], f32, tag="dnd")
                        nc.sync.dma_start(out=dnd, in_=dn[64:65, :])
                        rp = rowp.tile([64, 512], f32, tag="rp")
                        bcast = bass.AP(
                            tensor=dnd.tensor, offset=dnd.offset,
                            ap=[[0, 64]] + [list(p) for p in dnd.ap[1:]])
                        nc.sync.dma_start(out=rp, in_=bcast)
                        if pb == 0:
                            nc.vector.tensor_mul(
                                oT[hp][0:64, j * 512:(j + 1) * 512],
                                pso[pb][0:64, :], rp)
                        else:
                            ob = obp.tile([64, 512], bf, tag="ob")
                            nc.vector.tensor_mul(ob, pso[pb][0:64, :], rp)
                            nc.sync.dma_start(
                                out=oT[hp][64:128, j * 512:(j + 1) * 512], in_=ob)

        # ---------------- phase 5: output projection -----------------------
        with tc.tile_pool(name="pop", bufs=4, space="PSUM") as pop, \
             tc.tile_pool(name="outs", bufs=4) as outs:
            for st in range(16):
                for jc in range(2):
                    ps = pop.tile([128, 512], f32, tag="po")
                    for rt in range(2):
                        nc.tensor.matmul(
                            ps,
                            oT[rt][:, st * 128:(st + 1) * 128],
                            woT[rt][:, jc * 512:(jc + 1) * 512],
                            start=(rt == 0), stop=(rt == 1))
                    ot_sb = outs.tile([128, 512], f32, tag="osb")
                    nc.scalar.copy(ot_sb, ps)
                    nc.sync.dma_start(
                        out=outp[st * 128:(st + 1) * 128, jc * 512:(jc + 1) * 512],
                        in_=ot_sb)

    nc.compile()
    return nc


def _get_nc():
    if "nc" not in _CACHE:
        _CACHE["nc"] = _build_nc()
    return _CACHE["nc"]


def kernel(x, Wq, bq, Wk, bk, Wv, bv, Wo, bo, mask, _trace=False):
    x = np.asarray(x, dtype=np.float32)
    nc = _get_nc()
    in_maps = []
    for c in range(N_CORES):
        b, g = divmod(c, 4)
        r0 = g * RL
        in_maps.append({
            "xb": np.ascontiguousarray(x[b]),
            "wq": np.ascontiguousarray(Wq[r0:r0 + RL]),
            "wk": np.ascontiguousarray(Wk[r0:r0 + RL]),
            "wv": np.ascontiguousarray(Wv[r0:r0 + RL]),
            "wo": np.ascontiguousarray(Wo[:, r0:r0 + RL]),
            "bq": np.ascontiguousarray(bq[r0:r0 + RL]),
            "bk": np.ascontiguousarray(bk[r0:r0 + RL]),
            "bv": np.ascontiguousarray(bv[r0:r0 + RL]),
        })
    res = run_bass_kernel_spmd(nc, in_maps, core_ids=list(range(N_CORES)),
                               trace=_trace)
    out = np.zeros((B, S, D), dtype=np.float32)
    for b in range(B):
        acc = np.zeros((S, D), dtype=np.float32)
        for g in range(4):
            acc += res.results[4 * b + g]["outp"]
        out[b] = acc + np.asarray(bo, dtype=np.float32)[None, :]
    if _trace:
        return out, res
    return out


# revision 9
# speedup vs baseline: 1.2962x; 1.1331x over previous
"""Multi-head causal self-attention on 8 trn2 NeuronCores.

Sharding: data-parallel over batch (B=2) x tensor-parallel over heads
(16 heads -> 4 per core). Each core computes, for its (batch, 4-head
group): QKV projections (rows of Wq/Wk/Wv), causal attention, and a
partial output projection against its column slice of Wo. The host
sums the 4 partials per batch and adds bo.

Device kernel layout notes (per core):
  xT    : x^T         [D(part) x S]   8 tiles [128, 2048]
  qT/kT : (head dims)(part) x S, 2 tiles [128, 2048] (2 heads each)
  v_aug : [S(part) x (4*65)]  v columns + ones column per head (the
          ones column makes the attn@V matmul also emit the softmax
          denominator as output row 64)
  scoresT[t, s] = K Q^T blocks -> exp on ACT -> attn@V accumulation.
  Causal: only lower (t <= s) blocks are computed; diagonal blocks are
  masked with an on-device upper-triangular multiplicative mask.
All matmuls run as float32r (full-rate fp32 PE path, N>=256).
"""

import numpy as np

import concourse.bass as bass
import concourse.mybir as mybir
import concourse.tile as tile
from concourse import bacc
from concourse.bass_utils import run_bass_kernel_spmd
from concourse.masks import make_identity, make_upper_triangular

B, S, D, H = 2, 2048, 1024, 16
DK = D // H           # 64
HL = 4                # heads per core
RL = HL * DK          # 256 local head-dim rows
N_CORES = 8

f32 = mybir.dt.float32
f32r = mybir.dt.float32r
bf = mybir.dt.bfloat16
EXP = mybir.ActivationFunctionType.Exp
SCALE = 1.0 / np.sqrt(DK).astype(np.float32).item()

_CACHE: dict = {}


def _build_nc():
    from contextlib import ExitStack

    nc = bacc.Bacc("TRN2", target_bir_lowering=False, debug=False)

    xb = nc.dram_tensor("xb", [S, D], f32, kind="ExternalInput").ap()
    wq = nc.dram_tensor("wq", [RL, D], f32, kind="ExternalInput").ap()
    wk = nc.dram_tensor("wk", [RL, D], f32, kind="ExternalInput").ap()
    wv = nc.dram_tensor("wv", [RL, D], f32, kind="ExternalInput").ap()
    wo = nc.dram_tensor("wo", [D, RL], f32, kind="ExternalInput").ap()
    bq = nc.dram_tensor("bq", [RL], f32, kind="ExternalInput").ap()
    bk = nc.dram_tensor("bk", [RL], f32, kind="ExternalInput").ap()
    bv = nc.dram_tensor("bv", [RL], f32, kind="ExternalInput").ap()
    outp = nc.dram_tensor("outp", [S, D], f32, kind="ExternalOutput").ap()

    with tile.TileContext(nc) as tc, ExitStack() as ctx:
        const = ctx.enter_context(tc.tile_pool(name="const", bufs=1))
        ident = const.tile([128, 128], f32)
        make_identity(nc, ident)
        tri = const.tile([128, 128], bf)
        make_upper_triangular(nc, tri, val=1.0, diag=True)
        ones = const.tile([1, 512], bf)
        nc.vector.memset(ones, 1.0)
        bq_sb = const.tile([1, RL], bf)
        bk_sb = const.tile([1, RL], bf)
        bv_sb = const.tile([1, RL], bf)
        for b_dram, b_sb, nm in ((bq, bq_sb, "blq"), (bk, bk_sb, "blk"),
                                 (bv, bv_sb, "blv")):
            bl = const.tile([1, RL], f32, name=nm)
            nc.sync.dma_start(out=bl, in_=b_dram.rearrange("(a b) -> a b", a=1))
            nc.vector.tensor_copy(b_sb, bl)

        pers = ctx.enter_context(tc.tile_pool(name="pers", bufs=1))
        qT = [pers.tile([128, S], f32r, tag=f"qT{m}", name=f"qT{m}") for m in range(2)]
        kT = [pers.tile([128, S], f32r, tag=f"kT{m}", name=f"kT{m}") for m in range(2)]
        vaug = [pers.tile([128, HL * 65], bf, tag=f"va{t}", name=f"va{t}") for t in range(16)]
        oT = [pers.tile([128, S], bf, tag=f"oT{m}", name=f"oT{m}") for m in range(2)]
        woT = [pers.tile([128, D], bf, tag=f"woT{r}", name=f"woT{r}") for r in range(2)]
        wqT = [pers.tile([128, RL], bf, tag=f"wqT{k}", name=f"wqT{k}") for k in range(8)]
        wkT = [pers.tile([128, RL], bf, tag=f"wkT{k}", name=f"wkT{k}") for k in range(8)]
        wvT = [pers.tile([128, RL], bf, tag=f"wvT{k}", name=f"wvT{k}") for k in range(8)]

        # ---------------- phase 1-3: transposes + QKV projections ----------
        with tc.tile_pool(name="xin", bufs=3) as xin, \
             tc.tile_pool(name="win", bufs=2) as win, \
             tc.tile_pool(name="xTp", bufs=1) as xTp, \
             tc.tile_pool(name="tps", bufs=4, space="PSUM") as tps, \
             tc.tile_pool(name="mmp", bufs=4, space="PSUM") as mmp:
            xT = [xTp.tile([128, S], bf, tag=f"xT{k}", name=f"xT{k}") for k in range(8)]

            for wdram, wTl in ((wq, wqT), (wk, wkT), (wv, wvT)):
                for mt in range(2):
                    wt = win.tile([128, D], f32, tag="wload")
                    nc.sync.dma_start(out=wt, in_=wdram[mt * 128:(mt + 1) * 128, :])
                    for k in range(8):
                        pt = tps.tile([128, 128], f32, tag="tp")
                        nc.tensor.transpose(pt, wt[:, k * 128:(k + 1) * 128], ident)
                        nc.vector.tensor_copy(wTl[k][:, mt * 128:(mt + 1) * 128], pt)
            for jt in range(8):
                wt = win.tile([128, RL], f32, tag="wload")
                nc.sync.dma_start(out=wt, in_=wo[jt * 128:(jt + 1) * 128, :])
                for rt in range(2):
                    pt = tps.tile([128, 128], f32, tag="tp")
                    nc.tensor.transpose(pt, wt[:, rt * 128:(rt + 1) * 128], ident)
                    nc.vector.tensor_copy(woT[rt][:, jt * 128:(jt + 1) * 128], pt)
            for st in range(16):
                xt_ = xin.tile([128, D], f32, tag="xload")
                nc.sync.dma_start(out=xt_, in_=xb[st * 128:(st + 1) * 128, :])
                for k in range(8):
                    pt = tps.tile([128, 128], f32, tag="tp")
                    nc.tensor.transpose(pt, xt_[:, k * 128:(k + 1) * 128], ident)
                    nc.vector.tensor_copy(xT[k][:, st * 128:(st + 1) * 128], pt)

            # qT / kT: out[m(=2 heads' dims), s] accumulated over k-tiles
            for wTl, b_sb, dst in ((wqT, bq_sb, qT), (wkT, bk_sb, kT)):
                for mt in range(2):
                    for sc in range(4):
                        ps = mmp.tile([128, 512], f32, tag="pqk")
                        for k in range(8):
                            nc.tensor.matmul(
                                ps,
                                wTl[k][:, mt * 128:(mt + 1) * 128],
                                xT[k][:, sc * 512:(sc + 1) * 512],
                                start=(k == 0), stop=False)
                        nc.tensor.matmul(
                            ps,
                            b_sb[:, mt * 128:(mt + 1) * 128],
                            ones,
                            start=False, stop=True)
                        nc.vector.tensor_copy(dst[mt][:, sc * 512:(sc + 1) * 512], ps)

            # v: out[s, r] accumulated over k-tiles; scattered into v_aug
            for st in range(16):
                ps = mmp.tile([128, RL], f32, tag="pqk", name="pv")
                for k in range(8):
                    nc.tensor.matmul(
                        ps,
                        xT[k][:, st * 128:(st + 1) * 128],
                        wvT[k],
                        start=(k == 0), stop=False)
                nc.tensor.matmul(
                    ps, ones[:, 0:128], bv_sb,
                    start=False, stop=True)
                va3 = vaug[st].rearrange("p (h e) -> p h e", e=65)
                nc.vector.memset(va3[:, :, 64:65], 1.0)
                nc.scalar.copy(
                    va3[:, :, 0:64], ps.rearrange("p (h e) -> p h e", e=64))

        # ---------------- phase 4: attention ------------------------------
        with tc.tile_pool(name="expp", bufs=6) as expp, \
             tc.tile_pool(name="rowp", bufs=4) as rowp, \
             tc.tile_pool(name="drp", bufs=4, space="DRAM") as drp, \
             tc.tile_pool(name="obp", bufs=3) as obp, \
             tc.tile_pool(name="scp", bufs=3, space="PSUM") as scp, \
             tc.tile_pool(name="otp", bufs=4, space="PSUM") as otp:
            for hp in range(2):
                for j in range(4):
                    i_max = 4 * j + 3
                    pso = {0: otp.tile([65, 512], f32, tag="ot", name="otA"),
                           64: otp.tile([65, 512], f32, tag="ot", name="otB")}
                    for i in range(i_max + 1):
                        d = i - 4 * j  # >= 0: diagonal block index
                        for pb in (0, 64):
                            h_loc = 2 * hp + (pb // 64)
                            sc_ps = scp.tile([128, 512], f32, tag="sc")
                            nc.tensor.matmul(
                                sc_ps,
                                kT[hp][pb:pb + 64, i * 128:(i + 1) * 128],
                                qT[hp][pb:pb + 64, j * 512:(j + 1) * 512],
                                start=True, stop=True)
                            et = expp.tile([128, 512], bf, tag="e")
                            if d < 0:
                                nc.scalar.activation(et, sc_ps, EXP, scale=SCALE)
                            else:
                                off = 128 * d
                                if off > 0:
                                    nc.gpsimd.memset(et[:, 0:off], 0.0)
                                nc.scalar.activation(
                                    et[:, off:512], sc_ps[:, off:512], EXP,
                                    scale=SCALE)
                                nc.vector.tensor_mul(
                                    et[:, off:off + 128], et[:, off:off + 128], tri)
                            nc.tensor.matmul(
                                pso[pb],
                                vaug[i][:, h_loc * 65:(h_loc + 1) * 65],
                                et,
                                start=(i == 0), stop=(i == i_max))
                    # softmax divide: row 64 of pso holds the denominators
                    for pb in (0, 64):
                        # reciprocal at the same partition (64), then bounce
                        # through DRAM to broadcast across 64 partitions
                        dn = rowp.tile([65, 512], f32, tag="dn")
                        nc.vector.reciprocal(dn[64:65, :], pso[pb][64:65, :])
                        dnd = drp.tile([1, 512], f32, tag="dnd")
                        nc.sync.dma_start(out=dnd, in_=dn[64:65, :])
                        rp = rowp.tile([64, 512], f32, tag="rp")
                        bcast = bass.AP(
                            tensor=dnd.tensor, offset=dnd.offset,
                            ap=[[0, 64]] + [list(p) for p in dnd.ap[1:]])
                        nc.sync.dma_start(out=rp, in_=bcast)
                        if pb == 0:
                            nc.vector.tensor_mul(
                                oT[hp][0:64, j * 512:(j + 1) * 512],
                                pso[pb][0:64, :], rp)
                        else:
                            ob = obp.tile([64, 512], bf, tag="ob")
                            nc.vector.tensor_mul(ob, pso[pb][0:64, :], rp)
                            nc.sync.dma_start(
                                out=oT[hp][64:128, j * 512:(j + 1) * 512], in_=ob)

        # ---------------- phase 5: output projection -----------------------
        with tc.tile_pool(name="pop", bufs=4, space="PSUM") as pop, \
             tc.tile_pool(name="outs", bufs=4) as outs:
            for st in range(16):
                for jc in range(2):
                    ps = pop.tile([128, 512], f32, tag="po")
                    for rt in range(2):
                        nc.tensor.matmul(
                            ps,
                            oT[rt][:, st * 128:(st + 1) * 128],
                            woT[rt][:, jc * 512:(jc + 1) * 512],
                            start=(rt == 0), stop=(rt == 1))
                    ot_sb = outs.tile([128, 512], f32, tag="osb")
                    nc.scalar.copy(ot_sb, ps)
                    nc.sync.dma_start(
                        out=outp[st * 128:(st + 1) * 128, jc * 512:(jc + 1) * 512],
                        in_=ot_sb)

    nc.compile()
    return nc


def _get_nc():
    if "nc" not in _CACHE:
        _CACHE["nc"] = _build_nc()
    return _CACHE["nc"]


def kernel(x, Wq, bq, Wk, bk, Wv, bv, Wo, bo, mask, _trace=False):
    x = np.asarray(x, dtype=np.float32)
    nc = _get_nc()
    in_maps = []
    for c in range(N_CORES):
        b, g = divmod(c, 4)
        r0 = g * RL
        in_maps.append({
            "xb": np.ascontiguousarray(x[b]),
            "wq": np.ascontiguousarray(Wq[r0:r0 + RL]),
            "wk": np.ascontiguousarray(Wk[r0:r0 + RL]),
            "wv": np.ascontiguousarray(Wv[r0:r0 + RL]),
            "wo": np.ascontiguousarray(Wo[:, r0:r0 + RL]),
            "bq": np.ascontiguousarray(bq[r0:r0 + RL]),
            "bk": np.ascontiguousarray(bk[r0:r0 + RL]),
            "bv": np.ascontiguousarray(bv[r0:r0 + RL]),
        })
    res = run_bass_kernel_spmd(nc, in_maps, core_ids=list(range(N_CORES)),
                               trace=_trace)
    out = np.zeros((B, S, D), dtype=np.float32)
    for b in range(B):
        acc = np.zeros((S, D), dtype=np.float32)
        for g in range(4):
            acc += res.results[4 * b + g]["outp"]
        out[b] = acc + np.asarray(bo, dtype=np.float32)[None, :]
    if _trace:
        return out, res
    return out
